# revision 6
# baseline (speedup 1.0000x reference)
"""GQA attention (B=2, S=2048, DIM=2048, H=16, KVH=4, HD=128, RoPE, causal)
on 8 TRN2 NeuronCores.

Sharding: core c -> batch b = c//4, head-group g = c%4 (q heads 4g..4g+3,
which map exactly to kv head g). Each core computes the partial output
attn_heads @ wo_slice.T  ([S, DIM]); the host sums the 4 partials per batch.

Device layout (everything "transposed", feature-major; DRAM sources are kept
contiguous in transfer order — strided per-partition DRAM layouts measure
10-30x slower per descriptor on HW):
  xT   [DIM, S]   bf16   x[b].T
  wqT  [DIM, 512] bf16   (per-head even/odd-permuted, 1/sqrt(HD)-scaled) wq.T
  wkT  [DIM, 128] bf16   permuted wk.T
  wvT  [DIM, 128] bf16   wv.T (not permuted; v is not roped)
  woT  [512, DIM] bf16   wo[:, cols].T
  cosT [128, S]   bf16   [cos; cos] rope table, frequency-major, duplicated
  sinT [128, S]   bf16   [-sin; sin] sign-folded rope table

The per-head even/odd permutation (rows [0,2,..,126,1,3,..,127]) turns RoPE
pair-interleaving into contiguous half-partitions; q.k dot products are
invariant because q and k are permuted identically.

Attention is computed in transposed score layout: scoresT[k, q] so that
probsT feeds the PV matmul directly (lhsT = v natural layout) and attnT
falls out in [hd, q] = exactly the lhsT the output projection needs.

Schedule notes (v3):
  - DMA issue is split across BOTH HWDGE queues: Sync streams xT (dt
    singles first for fast first-arrival, then pairs, depth-2 completion
    chain = 2 transfers in flight); Scalar issues all weights (wk + wq
    head-0 immediately, rest depth-2 chained). PE work can start ~10us in.
  - HAM warm-up matmuls are gated on GpSimd memsets (GpSimd's sequencer
    wakes several us before DVE's), ramping the PE clock during the DMA
    lead-in.
  - Phase A ping-pongs PSUM pools (K/Q1/Q3 -> pool A, Q0/Q2/V -> pools
    At+Op) so each projection's psums are freed by ropes one step ahead of
    the next projection's needs; V-proj is never serialized behind Q3's
    ropes, and pool A is free for the first attention chunk that overlaps
    V-proj.
  - Attention q-chunks run SHORTEST-FIRST [0,512,1024,1536]: the shallow
    chunk-0 pipeline overlaps V-proj, each chunk's output projection
    (pure PE) fills the PE idle of the next chunk's ACT-bound score
    stream, and the final chunk's O-proj is a dense PE-only tail.
  - Engine balance in phase B (PE is the floor at ~125us): exp() is
    ACT-only (~85us); softmax-sum accumulation is split DVE 2/3 : GpSimd
    1/3; O-proj PSUM->SBUF copies are split ACT/DVE per-chunk so neither
    engine exceeds the PE time of its overlap period.
  - PSUM: scores rotate 4 bufs (tag sc), PV accumulators double-buffer
    (tag at), z + O-proj groups share 2 bufs (tag op).
"""

import math
import sys

import numpy as np

try:
    import concourse.bacc as bacc  # noqa: F401
except ImportError:
    sys.path.insert(0, "/opt/trn_rl_repo")

import ml_dtypes
import concourse.bacc as bacc
import concourse.tile as tile
from concourse import mybir
from concourse.bass_utils import run_bass_kernel_spmd
from concourse.bass import _add_dep_helper

BF16 = mybir.dt.bfloat16
F32 = mybir.dt.float32

B, S, DIM = 2, 2048, 2048
H, KVH, HD = 16, 4, 128
N_CORES = 8
P = 128
D_T = DIM // P      # 16 contraction tiles
NH = H // KVH       # 4 q-heads per core
QC = 512            # q-chunk (matmul moving free dim)
QB = S // QC        # 4 q-chunks
S_T = S // P        # 16 s-tiles / k-tiles
N_WARM = 10         # dummy warm-up matmuls to ramp HAM

_cached = {}


def _build_nc():
    nc = bacc.Bacc("TRN2", target_bir_lowering=False, debug=False,
                   num_devices=N_CORES)
    xT = nc.dram_tensor("xT", [DIM, S], BF16, kind="ExternalInput").ap()
    wqp = nc.dram_tensor("wqp", [NH, P, D_T, HD], BF16,
                         kind="ExternalInput").ap()
    wkp = nc.dram_tensor("wkp", [P, D_T, HD], BF16, kind="ExternalInput").ap()
    wvp = nc.dram_tensor("wvp", [P, D_T, HD], BF16, kind="ExternalInput").ap()
    wop = nc.dram_tensor("wop", [P, NH, DIM], BF16, kind="ExternalInput").ap()
    cosT = nc.dram_tensor("cosT", [HD, S], BF16, kind="ExternalInput").ap()
    sinT = nc.dram_tensor("sinT", [HD, S], BF16, kind="ExternalInput").ap()
    out = nc.dram_tensor("out", [S, DIM], BF16, kind="ExternalOutput").ap()

    with tile.TileContext(nc) as tc:
        _build_kernel(tc, xT, wqp, wkp, wvp, wop, cosT, sinT, out)
    nc.compile()
    return nc


def _build_kernel(tc, xT, wqp, wkp, wvp, wop, cosT, sinT, out):
    nc = tc.nc
    Exp = mybir.ActivationFunctionType.Exp

    with (
        tc.tile_pool(name="const", bufs=1) as const,
        tc.tile_pool(name="big", bufs=1) as big,
        tc.tile_pool(name="rtmp", bufs=8) as rtmp,
        tc.tile_pool(name="probs", bufs=9) as probs_pool,
        tc.tile_pool(name="pracc", bufs=3) as pracc_pool,
        tc.tile_pool(name="attn", bufs=6) as attn_pool,
        tc.tile_pool(name="rz", bufs=3) as rz_pool,
        tc.tile_pool(name="osb", bufs=2) as osb_pool,
        tc.tile_pool(name="psA", bufs=4, space="PSUM") as psA,
        tc.tile_pool(name="psAt", bufs=2, space="PSUM") as psAt,
        tc.tile_pool(name="psOp", bufs=2, space="PSUM") as psOp,
    ):
        # ---- constants + HAM warm-up ----
        # memsets on GpSimd: its sequencer wakes earliest, so the dummy
        # matmuls (which only need `dum` initialized) ramp the PE clock
        # governor during the DMA lead-in.
        ones = const.tile([P, P], BF16, name="ones")
        nc.gpsimd.memset(ones, 1.0)
        dum = const.tile([P, QC], BF16, name="dum")
        nc.gpsimd.memset(dum, 0.25)
        warm_ps = psA.tile([P, QC], F32, name="sc")
        for _ in range(N_WARM):
            nc.tensor.matmul(warm_ps[:, 0:384], lhsT=dum[:, 0:P],
                             rhs=dum[:, 0:384], start=True, stop=True)

        # ---- input DMAs ----
        # Sync queue: the xT stream. dt0/dt1 as singles (first data arrives
        # in ~half the time), then pairs; depth-2 completion chain so ~2
        # transfers are in flight and arrive in consumption order.
        xt_sb = big.tile([P, D_T, S], BF16, name="xt")
        xt_tiles = {}
        for dt in range(D_T):
            for sc in range(QB):
                xt_tiles[(dt, sc)] = xt_sb[:, dt, sc * QC:(sc + 1) * QC]
        groups = [(0, 1), (1, 2)] + [(d, d + 2) for d in range(2, D_T, 2)]
        xp_dmas = []
        for gi, (d0, d1) in enumerate(groups):
            dma = nc.sync.dma_start(
                out=xt_sb[:, d0:d1, :],
                in_=xT[d0 * P:d1 * P, :].rearrange("(t p) s -> p t s", p=P))
            if gi >= 2:
                _add_dep_helper(dma.ins, xp_dmas[gi - 2].ins, sync=True,
                                reason="stagger xT load")
            xp_dmas.append(dma)

        # Scalar (ACT) HWDGE queue: weights, all host-pre-arranged to dense
        # per-partition layouts (128 x 4KB+ descriptors, contiguous DRAM
        # source). The 16 DMA engines process descriptors of all transfers
        # in FIFO enqueue order, so late weights are chained onto the xT
        # stream's completions to keep the x tiles strictly ahead.
        wk_sb = big.tile([P, D_T, HD], BF16, name="wk")
        d_wk = nc.scalar.dma_start(out=wk_sb, in_=wkp)
        wq_sb = big.tile([P, NH, D_T, HD], BF16, name="wq")
        d_wq0 = nc.scalar.dma_start(out=wq_sb[:, 0], in_=wqp[0])
        wv_sb = big.tile([P, D_T, HD], BF16, name="wv")
        wo_sb = big.tile([P, NH, DIM], BF16, name="wo")
        cos_sb = const.tile([HD, S], BF16, name="cos")
        sin_sb = const.tile([HD, S], BF16, name="sin")
        for hh in (1, 2, 3):
            d = nc.scalar.dma_start(out=wq_sb[:, hh], in_=wqp[hh])
            _add_dep_helper(d.ins, xp_dmas[hh].ins, sync=True,
                            reason="wq head behind x stream")
        for dst, dsrc, gate in ((cos_sb, cosT, 4), (sin_sb, sinT, 4),
                                (wv_sb, wvp, 6), (wo_sb, wop, 7)):
            d = nc.scalar.dma_start(out=dst, in_=dsrc)
            _add_dep_helper(d.ins, xp_dmas[gate].ins, sync=True,
                            reason="late weight behind x stream")

        qT = big.tile([P, NH, S], BF16, name="qT")
        kT = big.tile([P, S], BF16, name="kT")
        v_sb = big.tile([P, S_T, HD], BF16, name="v")

        def rope(dst, ps, sc):
            """dst (bf16 [128,512] slice) <- rotate(ps).

            ACT stages ps to bf16 SBUF twice (straight + halves swapped via
            ScalarE partition-shifting copies); DVE then runs three
            full-width ops against the sign-folded tables:
            dst = st*[cos;cos] + sw*[-sin;sin]."""
            h = HD // 2
            st = rtmp.tile([P, QC], BF16, name="rst")
            sw = rtmp.tile([P, QC], BF16, name="rsw")
            nc.scalar.copy(out=st, in_=ps)
            nc.scalar.copy(out=sw[0:h, :], in_=ps[h:P, :])
            nc.scalar.copy(out=sw[h:P, :], in_=ps[0:h, :])
            cos_c = cos_sb[:, sc * QC:(sc + 1) * QC]
            sin_c = sin_sb[:, sc * QC:(sc + 1) * QC]
            t0 = rtmp.tile([P, QC], BF16, name="rt")
            t1 = rtmp.tile([P, QC], BF16, name="rt")
            nc.vector.tensor_mul(t0, st, cos_c)
            nc.vector.tensor_mul(t1, sw, sin_c)
            nc.vector.tensor_add(dst, t0, t1)

        # ---- K projection + Q head-0, dt-outer ----
        # K runs 4 dt-tiles ahead of Q-h0 so the PE starts as soon as the
        # first xT tile lands (wq0 arrives about then on the other queue).
        # K -> pool A, Q0 -> pools At+Op.
        kps = [psA.tile([P, QC], F32, name="sc") for _ in range(QB)]
        q0ps = [psAt.tile([P, QC], F32, name="at"),
                psAt.tile([P, QC], F32, name="at"),
                psOp.tile([P, QC], F32, name="op"),
                psOp.tile([P, QC], F32, name="op")]

        def kmm(dt):
            for sc in range(QB):
                nc.tensor.matmul(kps[sc], lhsT=wk_sb[:, dt, :],
                                 rhs=xt_tiles[(dt, sc)],
                                 start=(dt == 0), stop=(dt == D_T - 1))

        def q0mm(dt):
            for sc in range(QB):
                nc.tensor.matmul(q0ps[sc], lhsT=wq_sb[:, 0, dt, :],
                                 rhs=xt_tiles[(dt, sc)],
                                 start=(dt == 0), stop=(dt == D_T - 1))

        for dt in range(4):
            kmm(dt)
        for dt in range(4, D_T):
            kmm(dt)
            q0mm(dt - 4)
        for dt in range(D_T - 4, D_T):
            q0mm(dt)

        # K ropes first: K's psums (pool A, which Q1 needs) finished 4
        # dt-steps before q0's, so they drain while q0's last matmuls run.
        for sc in range(QB):
            rope(kT[:, sc * QC:(sc + 1) * QC], kps[sc], sc)
        for sc in range(QB):
            rope(qT[:, 0, sc * QC:(sc + 1) * QC], q0ps[sc], sc)

        # ---- Q heads 1..3, dt-outer per head, ping-ponging pools ----
        for hh in range(1, NH):
            if hh % 2 == 1:
                qps = [psA.tile([P, QC], F32, name="sc") for _ in range(QB)]
            else:
                qps = [psAt.tile([P, QC], F32, name="at"),
                       psAt.tile([P, QC], F32, name="at"),
                       psOp.tile([P, QC], F32, name="op"),
                       psOp.tile([P, QC], F32, name="op")]
            for dt in range(D_T):
                for sc in range(QB):
                    nc.tensor.matmul(
                        qps[sc], lhsT=wq_sb[:, hh, dt, :],
                        rhs=xt_tiles[(dt, sc)],
                        start=(dt == 0), stop=(dt == D_T - 1))
            for sc in range(QB):
                rope(qT[:, hh, sc * QC:(sc + 1) * QC], qps[sc], sc)

        # ---- V projection (natural [s, hd] layout) ----
        # Pools At/Op (freed by Q2's ropes long before); pool A stays free
        # for the chunk-0 attention stream that overlaps V-proj.
        for st in range(S_T):
            ps = psA.tile([P, QC], F32, name="sc")
            for dt in range(D_T):
                nc.tensor.matmul(
                    ps[:, 0:HD],
                    lhsT=xt_tiles[(dt, st // 4)][:, (st % 4) * P:(st % 4 + 1) * P],
                    rhs=wv_sb[:, dt, :],
                    start=(dt == 0), stop=(dt == D_T - 1))
            nc.vector.tensor_copy(out=v_sb[:, st, :], in_=ps[:, 0:HD])

        # ---- attention + output projection, per q-chunk ----
        # Chunks run SHORTEST-first: the shallow chunk-0 pipeline overlaps
        # V-proj, each chunk's O-proj (pure PE) fills the PE idle time of
        # the next chunk's ACT-bound score stream, and the final (longest)
        # chunk leaves a dense PE-only O-proj tail.
        # O-proj copy engine split per chunk: the fraction on ACT shrinks
        # when the overlapping score stream is ACT-heavy.
        chunks = [(0, 512), (512, 512), (1024, 512), (1536, 512)]
        # index (st*4+dc)%4 -> True=ACT copy, False=DVE copy, per chunk
        copy_act = {
            0: (True, True, True, False),   # overlaps chunk-512 stream
            1: (True, True, False, False),  # overlaps chunk-1024 stream
            2: (True, False, False, False),  # overlaps chunk-1536 (ACT-hot)
            3: (True, False, True, False),  # standalone tail
        }
        for ci, (q0, qw) in enumerate(chunks):
            nk = (q0 + qw) // P  # causal k-tiles for this q-chunk
            attn_tiles = []
            for hh in range(NH):
                at_ps = psAt.tile([P, qw], F32, name="at")
                pr_acc = pracc_pool.tile([P, qw], BF16, name="pracc")
                for k in range(nk):
                    # On diagonal tiles only columns q0+off.. are causally
                    # valid; narrow every stage to that width.
                    off = max(0, k * P - q0)
                    w = qw - off
                    diag = k * P >= q0
                    sc_ps = psA.tile([P, QC], F32, name="sc")
                    nc.tensor.matmul(sc_ps[:, 0:w], lhsT=kT[:, k * P:(k + 1) * P],
                                     rhs=qT[:, hh, q0 + off:q0 + qw],
                                     start=True, stop=True)
                    if k == 0:
                        # exp lands directly in the softmax-sum accumulator
                        pr = pr_acc
                    else:
                        pr = probs_pool.tile([P, QC], BF16, name="pr")
                    nc.scalar.activation(out=pr[:, 0:w], in_=sc_ps[:, 0:w],
                                         func=Exp)
                    if diag:  # zero where c' < r
                        nc.gpsimd.affine_select(
                            out=pr[:, 0:w], in_=pr[:, 0:w],
                            compare_op=mybir.AluOpType.is_ge,
                            fill=0.0, base=0, pattern=[[1, w]],
                            channel_multiplier=-1)
                    nc.tensor.matmul(at_ps[:, off:qw], lhsT=v_sb[:, k, :],
                                     rhs=pr[:, 0:w],
                                     start=(k == 0), stop=(k == nk - 1))
                    if k > 0:
                        nc.vector.tensor_add(pr_acc[:, off:qw],
                                             pr_acc[:, off:qw], pr[:, 0:w])
                z_ps = psOp.tile([P, qw], F32, name="op")
                nc.tensor.matmul(z_ps, lhsT=ones, rhs=pr_acc,
                                 start=True, stop=True)
                rz = rz_pool.tile([P, qw], F32, name="rz")
                nc.vector.reciprocal_approx_fast(out=rz, in_=z_ps)
                a_sb = attn_pool.tile([P, qw], BF16, name="attn")
                nc.vector.tensor_mul(a_sb, at_ps, rz)
                attn_tiles.append(a_sb)

            # Output projection for this chunk; all output DMAs issue from
            # Sync (idle in phase B; ACT must stay mostly exp-only).
            for st in range(qw // P):
                row0 = q0 + st * P
                o_sb = osb_pool.tile([P, DIM], BF16, name="osb")
                for dc in range(DIM // QC):
                    op_ps = psOp.tile([P, QC], F32, name="op")
                    for j in range(NH):
                        nc.tensor.matmul(
                            op_ps, lhsT=attn_tiles[j][:, st * P:(st + 1) * P],
                            rhs=wo_sb[:, j, dc * QC:(dc + 1) * QC],
                            start=(j == 0), stop=(j == NH - 1))
                    if copy_act[ci][(st * 4 + dc) % 4]:
                        nc.scalar.copy(out=o_sb[:, dc * QC:(dc + 1) * QC],
                                       in_=op_ps)
                    else:
                        nc.vector.tensor_copy(out=o_sb[:, dc * QC:(dc + 1) * QC],
                                              in_=op_ps)
                    if dc == 1:
                        nc.sync.dma_start(out=out[row0:row0 + P, 0:2 * QC],
                                          in_=o_sb[:, 0:2 * QC])
                nc.sync.dma_start(out=out[row0:row0 + P, 2 * QC:DIM],
                                  in_=o_sb[:, 2 * QC:DIM])


def _get_nc():
    if "nc" not in _cached:
        _cached["nc"] = _build_nc()
    return _cached["nc"]


def _prep_in_maps(x, freqs_cis, wq, wk, wv, wo):
    bf = ml_dtypes.bfloat16
    perm = np.concatenate([np.arange(0, HD, 2), np.arange(1, HD, 2)])
    scale = 1.0 / math.sqrt(HD)
    wq_p = (wq.reshape(H, HD, DIM)[:, perm, :] * scale).astype(np.float32)
    wk_p = wk.reshape(KVH, HD, DIM)[:, perm, :]
    cos_h = np.ascontiguousarray(freqs_cis[:, :, 0].T)  # [64, S]
    sin_h = np.ascontiguousarray(freqs_cis[:, :, 1].T)
    cosT = np.concatenate([cos_h, cos_h], axis=0).astype(bf)   # [128, S]
    sinT = np.concatenate([-sin_h, sin_h], axis=0).astype(bf)

    in_maps = []
    for c in range(N_CORES):
        b, g = c // KVH, c % KVH
        hq = slice(NH * g, NH * (g + 1))
        def p_t_j(wT):  # [DIM, J] -> [P, D_T, J] dense per partition
            J = wT.shape[1]
            return np.ascontiguousarray(
                wT.reshape(D_T, P, J).transpose(1, 0, 2)).astype(bf)

        wq_core = wq_p[hq].reshape(NH * HD, DIM).T  # [DIM, NH*HD]
        wqp_h = np.ascontiguousarray(
            wq_core.reshape(D_T, P, NH, HD).transpose(2, 1, 0, 3)).astype(bf)
        wo_core = wo[:, NH * HD * g:NH * HD * (g + 1)].T  # [NH*HD, DIM]
        wop_h = np.ascontiguousarray(
            wo_core.reshape(NH, HD, DIM).transpose(1, 0, 2)).astype(bf)
        in_maps.append({
            "xT": np.ascontiguousarray(x[b].T).astype(bf),
            "wqp": wqp_h,
            "wkp": p_t_j(np.ascontiguousarray(wk_p[g].T)),
            "wvp": p_t_j(np.ascontiguousarray(wv[g * HD:(g + 1) * HD].T)),
            "wop": wop_h,
            "cosT": cosT,
            "sinT": sinT,
        })
    return in_maps


def _reduce_outputs(results):
    out = np.zeros((B, S, DIM), np.float32)
    for c in range(N_CORES):
        out[c // KVH] += results[c]["out"].astype(np.float32)
    return out


def kernel(x, freqs_cis, wq, wk, wv, wo, _trace=False, _trace_kwargs=None):
    nc = _get_nc()
    x, freqs_cis, wq, wk, wv, wo = (
        np.asarray(a, np.float32) for a in (x, freqs_cis, wq, wk, wv, wo))
    in_maps = _prep_in_maps(x, freqs_cis, wq, wk, wv, wo)
    res = run_bass_kernel_spmd(nc, in_maps, core_ids=list(range(N_CORES)),
                               trace=_trace, **(_trace_kwargs or {}))
    out = _reduce_outputs(res.results)
    if _trace:
        _cached["last_exec_time_ns"] = res.exec_time_ns
        _cached["last_results"] = res
    return out


# revision 8
# speedup vs baseline: 1.0149x; 1.0149x over previous
"""GQA attention (B=2, S=2048, DIM=2048, H=16, KVH=4, HD=128, RoPE, causal)
on 8 TRN2 NeuronCores.

Sharding: core c -> batch b = c//4, head-group g = c%4 (q heads 4g..4g+3,
which map exactly to kv head g). Each core computes the partial output
attn_heads @ wo_slice.T  ([S, DIM]); the host sums the 4 partials per batch.

Device layout (everything "transposed", feature-major; DRAM sources are kept
contiguous in transfer order — strided per-partition DRAM layouts measure
10-30x slower per descriptor on HW):
  xT   [DIM, S]   bf16   x[b].T
  wqT  [DIM, 512] bf16   (per-head even/odd-permuted, 1/sqrt(HD)-scaled) wq.T
  wkT  [DIM, 128] bf16   permuted wk.T
  wvT  [DIM, 128] bf16   wv.T (not permuted; v is not roped)
  woT  [512, DIM] bf16   wo[:, cols].T
  cosT [128, S]   bf16   [cos; cos] rope table, frequency-major, duplicated
  sinT [128, S]   bf16   [-sin; sin] sign-folded rope table

The per-head even/odd permutation (rows [0,2,..,126,1,3,..,127]) turns RoPE
pair-interleaving into contiguous half-partitions; q.k dot products are
invariant because q and k are permuted identically.

Attention is computed in transposed score layout: scoresT[k, q] so that
probsT feeds the PV matmul directly (lhsT = v natural layout) and attnT
falls out in [hd, q] = exactly the lhsT the output projection needs.

Schedule notes (v3):
  - DMA issue is split across BOTH HWDGE queues: Sync streams xT (dt
    singles first for fast first-arrival, then pairs, depth-2 completion
    chain = 2 transfers in flight); Scalar issues all weights (wk + wq
    head-0 immediately, rest depth-2 chained). PE work can start ~10us in.
  - HAM warm-up matmuls are gated on GpSimd memsets (GpSimd's sequencer
    wakes several us before DVE's), ramping the PE clock during the DMA
    lead-in.
  - Phase A ping-pongs PSUM pools (K/Q1/Q3 -> pool A, Q0/Q2/V -> pools
    At+Op) so each projection's psums are freed by ropes one step ahead of
    the next projection's needs; V-proj is never serialized behind Q3's
    ropes, and pool A is free for the first attention chunk that overlaps
    V-proj.
  - Attention q-chunks run SHORTEST-FIRST [0,512,1024,1536]: the shallow
    chunk-0 pipeline overlaps V-proj, each chunk's output projection
    (pure PE) fills the PE idle of the next chunk's ACT-bound score
    stream, and the final chunk's O-proj is a dense PE-only tail.
  - Engine balance in phase B (PE is the floor at ~125us): exp() is
    ACT-only (~85us); softmax-sum accumulation is split DVE 2/3 : GpSimd
    1/3; O-proj PSUM->SBUF copies are split ACT/DVE per-chunk so neither
    engine exceeds the PE time of its overlap period.
  - PSUM: scores rotate 4 bufs (tag sc), PV accumulators double-buffer
    (tag at), z + O-proj groups share 2 bufs (tag op).
"""

import math
import sys

import numpy as np

try:
    import concourse.bacc as bacc  # noqa: F401
except ImportError:
    sys.path.insert(0, "/opt/trn_rl_repo")

import ml_dtypes
import concourse.bacc as bacc
import concourse.tile as tile
from concourse import mybir
from concourse.bass_utils import run_bass_kernel_spmd
from concourse.bass import _add_dep_helper

BF16 = mybir.dt.bfloat16
F32 = mybir.dt.float32

B, S, DIM = 2, 2048, 2048
H, KVH, HD = 16, 4, 128
N_CORES = 8
P = 128
D_T = DIM // P      # 16 contraction tiles
NH = H // KVH       # 4 q-heads per core
QC = 512            # q-chunk (matmul moving free dim)
QB = S // QC        # 4 q-chunks
S_T = S // P        # 16 s-tiles / k-tiles
N_WARM = 8          # dummy warm-up matmuls to ramp HAM

_cached = {}


def _build_nc():
    nc = bacc.Bacc("TRN2", target_bir_lowering=False, debug=False,
                   num_devices=N_CORES)
    xT = nc.dram_tensor("xT", [DIM, S], BF16, kind="ExternalInput").ap()
    wqp = nc.dram_tensor("wqp", [NH, P, D_T, HD], BF16,
                         kind="ExternalInput").ap()
    wkp = nc.dram_tensor("wkp", [P, D_T, HD], BF16, kind="ExternalInput").ap()
    wvp = nc.dram_tensor("wvp", [P, D_T, HD], BF16, kind="ExternalInput").ap()
    wop = nc.dram_tensor("wop", [P, NH, DIM], BF16, kind="ExternalInput").ap()
    cosT = nc.dram_tensor("cosT", [HD, S], BF16, kind="ExternalInput").ap()
    sinT = nc.dram_tensor("sinT", [HD, S], BF16, kind="ExternalInput").ap()
    out = nc.dram_tensor("out", [S, DIM], BF16, kind="ExternalOutput").ap()

    with tile.TileContext(nc) as tc:
        _build_kernel(tc, xT, wqp, wkp, wvp, wop, cosT, sinT, out)
    nc.compile()
    return nc


def _build_kernel(tc, xT, wqp, wkp, wvp, wop, cosT, sinT, out):
    nc = tc.nc
    Exp = mybir.ActivationFunctionType.Exp

    with (
        tc.tile_pool(name="const", bufs=1) as const,
        tc.tile_pool(name="big", bufs=1) as big,
        tc.tile_pool(name="rtmp", bufs=8) as rtmp,
        tc.tile_pool(name="probs", bufs=6) as probs_pool,
        tc.tile_pool(name="pracc", bufs=3) as pracc_pool,
        tc.tile_pool(name="attn", bufs=6) as attn_pool,
        tc.tile_pool(name="rz", bufs=3) as rz_pool,
        tc.tile_pool(name="osb", bufs=2) as osb_pool,
        tc.tile_pool(name="psA", bufs=4, space="PSUM") as psA,
        tc.tile_pool(name="psAt", bufs=2, space="PSUM") as psAt,
        tc.tile_pool(name="psOp", bufs=2, space="PSUM") as psOp,
    ):
        # ---- constants + HAM warm-up ----
        # memsets on GpSimd: its sequencer wakes earliest, so the dummy
        # matmuls (which only need `dum` initialized) ramp the PE clock
        # governor during the DMA lead-in.
        ones = const.tile([P, P], BF16, name="ones")
        nc.gpsimd.memset(ones, 1.0)
        dum = const.tile([P, QC], BF16, name="dum")
        nc.gpsimd.memset(dum, 0.25)
        warm_ps = psA.tile([P, QC], F32, name="sc")
        for _ in range(N_WARM):
            nc.tensor.matmul(warm_ps[:, 0:384], lhsT=dum[:, 0:P],
                             rhs=dum[:, 0:384], start=True, stop=True)

        # ---- input DMAs ----
        # Sync queue: the xT stream. dt0/dt1 as singles (first data arrives
        # in ~half the time), then pairs; depth-2 completion chain so ~2
        # transfers are in flight and arrive in consumption order.
        xt_sb = big.tile([P, D_T, S], BF16, name="xt")
        xt_tiles = {}
        for dt in range(D_T):
            for sc in range(QB):
                xt_tiles[(dt, sc)] = xt_sb[:, dt, sc * QC:(sc + 1) * QC]
        groups = [(0, 1), (1, 2)] + [(d, d + 2) for d in range(2, D_T, 2)]
        xp_dmas = []
        for gi, (d0, d1) in enumerate(groups):
            dma = nc.sync.dma_start(
                out=xt_sb[:, d0:d1, :],
                in_=xT[d0 * P:d1 * P, :].rearrange("(t p) s -> p t s", p=P))
            if gi >= 2:
                _add_dep_helper(dma.ins, xp_dmas[gi - 2].ins, sync=True,
                                reason="stagger xT load")
            xp_dmas.append(dma)

        # Scalar (ACT) HWDGE queue: weights, all host-pre-arranged to dense
        # per-partition layouts (128 x 4KB+ descriptors, contiguous DRAM
        # source). The 16 DMA engines process descriptors of all transfers
        # in FIFO enqueue order, so late weights are chained onto the xT
        # stream's completions to keep the x tiles strictly ahead.
        wk_sb = big.tile([P, D_T, HD], BF16, name="wk")
        d_wk = nc.scalar.dma_start(out=wk_sb, in_=wkp)
        wq_sb = big.tile([P, NH, D_T, HD], BF16, name="wq")
        d_wq0 = nc.scalar.dma_start(out=wq_sb[:, 0], in_=wqp[0])
        wv_sb = big.tile([P, D_T, HD], BF16, name="wv")
        wo_sb = big.tile([P, NH, DIM], BF16, name="wo")
        cos_sb = const.tile([HD, S], BF16, name="cos")
        sin_sb = const.tile([HD, S], BF16, name="sin")
        for hh in (1, 2, 3):
            d = nc.scalar.dma_start(out=wq_sb[:, hh], in_=wqp[hh])
            _add_dep_helper(d.ins, xp_dmas[hh].ins, sync=True,
                            reason="wq head behind x stream")
        for dst, dsrc, gate in ((cos_sb, cosT, 4), (sin_sb, sinT, 4),
                                (wv_sb, wvp, 6), (wo_sb, wop, 7)):
            d = nc.scalar.dma_start(out=dst, in_=dsrc)
            _add_dep_helper(d.ins, xp_dmas[gate].ins, sync=True,
                            reason="late weight behind x stream")

        qT = big.tile([P, NH, S], BF16, name="qT")
        kT = big.tile([P, S], BF16, name="kT")
        v_sb = big.tile([P, S_T, HD], BF16, name="v")

        def rope(dst, ps, sc):
            """dst (bf16 [128,512] slice) <- rotate(ps).

            ACT stages ps to bf16 SBUF twice (straight + halves swapped via
            ScalarE partition-shifting copies); DVE then runs three
            full-width ops against the sign-folded tables:
            dst = st*[cos;cos] + sw*[-sin;sin]."""
            h = HD // 2
            st = rtmp.tile([P, QC], BF16, name="rst")
            sw = rtmp.tile([P, QC], BF16, name="rsw")
            nc.scalar.copy(out=st, in_=ps)
            nc.scalar.copy(out=sw[0:h, :], in_=ps[h:P, :])
            nc.scalar.copy(out=sw[h:P, :], in_=ps[0:h, :])
            cos_c = cos_sb[:, sc * QC:(sc + 1) * QC]
            sin_c = sin_sb[:, sc * QC:(sc + 1) * QC]
            t0 = rtmp.tile([P, QC], BF16, name="rt")
            t1 = rtmp.tile([P, QC], BF16, name="rt")
            nc.vector.tensor_mul(t0, st, cos_c)
            nc.vector.tensor_mul(t1, sw, sin_c)
            nc.vector.tensor_add(dst, t0, t1)

        # ---- K projection + Q head-0, dt-outer ----
        # K runs 4 dt-tiles ahead of Q-h0 so the PE starts as soon as the
        # first xT tile lands (wq0 arrives about then on the other queue).
        # K -> pool A, Q0 -> pools At+Op.
        kps = [psA.tile([P, QC], F32, name="sc") for _ in range(QB)]
        q0ps = [psAt.tile([P, QC], F32, name="at"),
                psAt.tile([P, QC], F32, name="at"),
                psOp.tile([P, QC], F32, name="op"),
                psOp.tile([P, QC], F32, name="op")]

        def kmm(dt):
            for sc in range(QB):
                nc.tensor.matmul(kps[sc], lhsT=wk_sb[:, dt, :],
                                 rhs=xt_tiles[(dt, sc)],
                                 start=(dt == 0), stop=(dt == D_T - 1))

        def q0mm(dt):
            for sc in range(QB):
                nc.tensor.matmul(q0ps[sc], lhsT=wq_sb[:, 0, dt, :],
                                 rhs=xt_tiles[(dt, sc)],
                                 start=(dt == 0), stop=(dt == D_T - 1))

        for dt in range(4):
            kmm(dt)
        for dt in range(4, D_T):
            kmm(dt)
            q0mm(dt - 4)
        for dt in range(D_T - 4, D_T):
            q0mm(dt)

        # K ropes first: K's psums (pool A, which Q1 needs) finished 4
        # dt-steps before q0's, so they drain while q0's last matmuls run.
        for sc in range(QB):
            rope(kT[:, sc * QC:(sc + 1) * QC], kps[sc], sc)
        for sc in range(QB):
            rope(qT[:, 0, sc * QC:(sc + 1) * QC], q0ps[sc], sc)

        # ---- Q heads 1..3, dt-outer per head, ping-ponging pools ----
        for hh in range(1, NH):
            if hh % 2 == 1:
                qps = [psA.tile([P, QC], F32, name="sc") for _ in range(QB)]
            else:
                qps = [psAt.tile([P, QC], F32, name="at"),
                       psAt.tile([P, QC], F32, name="at"),
                       psOp.tile([P, QC], F32, name="op"),
                       psOp.tile([P, QC], F32, name="op")]
            for dt in range(D_T):
                for sc in range(QB):
                    nc.tensor.matmul(
                        qps[sc], lhsT=wq_sb[:, hh, dt, :],
                        rhs=xt_tiles[(dt, sc)],
                        start=(dt == 0), stop=(dt == D_T - 1))
            for sc in range(QB):
                rope(qT[:, hh, sc * QC:(sc + 1) * QC], qps[sc], sc)

        # ---- V projection (natural [s, hd] layout) ----
        # Pools At/Op (freed by Q2's ropes long before); pool A stays free
        # for the chunk-0 attention stream that overlaps V-proj.
        for st in range(S_T):
            if st % 2 == 0:
                ps = psAt.tile([P, QC], F32, name="at")
            else:
                ps = psOp.tile([P, QC], F32, name="op")
            for dt in range(D_T):
                nc.tensor.matmul(
                    ps[:, 0:HD],
                    lhsT=xt_tiles[(dt, st // 4)][:, (st % 4) * P:(st % 4 + 1) * P],
                    rhs=wv_sb[:, dt, :],
                    start=(dt == 0), stop=(dt == D_T - 1))
            nc.vector.tensor_copy(out=v_sb[:, st, :], in_=ps[:, 0:HD])

        # ---- attention + output projection, per q-chunk ----
        # Chunks run SHORTEST-first: the shallow chunk-0 pipeline overlaps
        # V-proj, each chunk's O-proj (pure PE) fills the PE idle time of
        # the next chunk's ACT-bound score stream, and the final (longest)
        # chunk leaves a dense PE-only O-proj tail.
        # O-proj copy engine split per chunk: the fraction on ACT shrinks
        # when the overlapping score stream is ACT-heavy.
        chunks = [(0, 512), (512, 512), (1024, 512), (1536, 512)]
        # index (st*4+dc)%4 -> True=ACT copy, False=DVE copy, per chunk
        copy_act = {
            0: (True, True, True, False),   # overlaps chunk-512 stream
            1: (True, True, False, False),  # overlaps chunk-1024 stream
            2: (True, False, False, False),  # overlaps chunk-1536 (ACT-hot)
            3: (True, False, True, False),  # standalone tail
        }
        for ci, (q0, qw) in enumerate(chunks):
            nk = (q0 + qw) // P  # causal k-tiles for this q-chunk
            attn_tiles = []
            for hh in range(NH):
                at_ps = psAt.tile([P, qw], F32, name="at")
                pr_acc = pracc_pool.tile([P, qw], BF16, name="pracc")
                for k in range(nk):
                    # On diagonal tiles only columns q0+off.. are causally
                    # valid; narrow every stage to that width.
                    off = max(0, k * P - q0)
                    w = qw - off
                    diag = k * P >= q0
                    sc_ps = psA.tile([P, QC], F32, name="sc")
                    nc.tensor.matmul(sc_ps[:, 0:w], lhsT=kT[:, k * P:(k + 1) * P],
                                     rhs=qT[:, hh, q0 + off:q0 + qw],
                                     start=True, stop=True)
                    if k == 0:
                        # exp lands directly in the softmax-sum accumulator
                        pr = pr_acc
                    else:
                        pr = probs_pool.tile([P, QC], BF16, name="pr")
                    nc.scalar.activation(out=pr[:, 0:w], in_=sc_ps[:, 0:w],
                                         func=Exp)
                    if diag:  # zero where c' < r
                        nc.gpsimd.affine_select(
                            out=pr[:, 0:w], in_=pr[:, 0:w],
                            compare_op=mybir.AluOpType.is_ge,
                            fill=0.0, base=0, pattern=[[1, w]],
                            channel_multiplier=-1)
                    nc.tensor.matmul(at_ps[:, off:qw], lhsT=v_sb[:, k, :],
                                     rhs=pr[:, 0:w],
                                     start=(k == 0), stop=(k == nk - 1))
                    if k > 0:
                        nc.vector.tensor_add(pr_acc[:, off:qw],
                                             pr_acc[:, off:qw], pr[:, 0:w])
                z_ps = psOp.tile([P, qw], F32, name="op")
                nc.tensor.matmul(z_ps, lhsT=ones, rhs=pr_acc,
                                 start=True, stop=True)
                rz = rz_pool.tile([P, qw], F32, name="rz")
                nc.vector.reciprocal_approx_fast(out=rz, in_=z_ps)
                a_sb = attn_pool.tile([P, qw], BF16, name="attn")
                nc.vector.tensor_mul(a_sb, at_ps, rz)
                attn_tiles.append(a_sb)

            # Output projection for this chunk; all output DMAs issue from
            # Sync (idle in phase B; ACT must stay mostly exp-only).
            for st in range(qw // P):
                row0 = q0 + st * P
                o_sb = osb_pool.tile([P, DIM], BF16, name="osb")
                for dc in range(DIM // QC):
                    op_ps = psOp.tile([P, QC], F32, name="op")
                    for j in range(NH):
                        nc.tensor.matmul(
                            op_ps, lhsT=attn_tiles[j][:, st * P:(st + 1) * P],
                            rhs=wo_sb[:, j, dc * QC:(dc + 1) * QC],
                            start=(j == 0), stop=(j == NH - 1))
                    if copy_act[ci][(st * 4 + dc) % 4]:
                        nc.scalar.copy(out=o_sb[:, dc * QC:(dc + 1) * QC],
                                       in_=op_ps)
                    else:
                        nc.vector.tensor_copy(out=o_sb[:, dc * QC:(dc + 1) * QC],
                                              in_=op_ps)
                    if dc == 1:
                        nc.sync.dma_start(out=out[row0:row0 + P, 0:2 * QC],
                                          in_=o_sb[:, 0:2 * QC])
                nc.sync.dma_start(out=out[row0:row0 + P, 2 * QC:DIM],
                                  in_=o_sb[:, 2 * QC:DIM])


def _get_nc():
    if "nc" not in _cached:
        _cached["nc"] = _build_nc()
    return _cached["nc"]


def _prep_in_maps(x, freqs_cis, wq, wk, wv, wo):
    bf = ml_dtypes.bfloat16
    perm = np.concatenate([np.arange(0, HD, 2), np.arange(1, HD, 2)])
    scale = 1.0 / math.sqrt(HD)
    wq_p = (wq.reshape(H, HD, DIM)[:, perm, :] * scale).astype(np.float32)
    wk_p = wk.reshape(KVH, HD, DIM)[:, perm, :]
    cos_h = np.ascontiguousarray(freqs_cis[:, :, 0].T)  # [64, S]
    sin_h = np.ascontiguousarray(freqs_cis[:, :, 1].T)
    cosT = np.concatenate([cos_h, cos_h], axis=0).astype(bf)   # [128, S]
    sinT = np.concatenate([-sin_h, sin_h], axis=0).astype(bf)

    in_maps = []
    for c in range(N_CORES):
        b, g = c // KVH, c % KVH
        hq = slice(NH * g, NH * (g + 1))
        def p_t_j(wT):  # [DIM, J] -> [P, D_T, J] dense per partition
            J = wT.shape[1]
            return np.ascontiguousarray(
                wT.reshape(D_T, P, J).transpose(1, 0, 2)).astype(bf)

        wq_core = wq_p[hq].reshape(NH * HD, DIM).T  # [DIM, NH*HD]
        wqp_h = np.ascontiguousarray(
            wq_core.reshape(D_T, P, NH, HD).transpose(2, 1, 0, 3)).astype(bf)
        wo_core = wo[:, NH * HD * g:NH * HD * (g + 1)].T  # [NH*HD, DIM]
        wop_h = np.ascontiguousarray(
            wo_core.reshape(NH, HD, DIM).transpose(1, 0, 2)).astype(bf)
        in_maps.append({
            "xT": np.ascontiguousarray(x[b].T).astype(bf),
            "wqp": wqp_h,
            "wkp": p_t_j(np.ascontiguousarray(wk_p[g].T)),
            "wvp": p_t_j(np.ascontiguousarray(wv[g * HD:(g + 1) * HD].T)),
            "wop": wop_h,
            "cosT": cosT,
            "sinT": sinT,
        })
    return in_maps


def _reduce_outputs(results):
    out = np.zeros((B, S, DIM), np.float32)
    for c in range(N_CORES):
        out[c // KVH] += results[c]["out"].astype(np.float32)
    return out


def kernel(x, freqs_cis, wq, wk, wv, wo, _trace=False, _trace_kwargs=None):
    nc = _get_nc()
    x, freqs_cis, wq, wk, wv, wo = (
        np.asarray(a, np.float32) for a in (x, freqs_cis, wq, wk, wv, wo))
    in_maps = _prep_in_maps(x, freqs_cis, wq, wk, wv, wo)
    res = run_bass_kernel_spmd(nc, in_maps, core_ids=list(range(N_CORES)),
                               trace=_trace, **(_trace_kwargs or {}))
    out = _reduce_outputs(res.results)
    if _trace:
        _cached["last_exec_time_ns"] = res.exec_time_ns
        _cached["last_results"] = res
    return out


# revision 9
# speedup vs baseline: 1.0233x; 1.0082x over previous
"""GQA attention (B=2, S=2048, DIM=2048, H=16, KVH=4, HD=128, RoPE, causal)
on 8 TRN2 NeuronCores.

Sharding: core c -> batch b = c//4, head-group g = c%4 (q heads 4g..4g+3,
which map exactly to kv head g). Each core computes the partial output
attn_heads @ wo_slice.T  ([S, DIM]); the host sums the 4 partials per batch.

Device layout (everything "transposed", feature-major; DRAM sources are kept
contiguous in transfer order — strided per-partition DRAM layouts measure
10-30x slower per descriptor on HW):
  xT   [DIM, S]   bf16   x[b].T
  wqT  [DIM, 512] bf16   (per-head even/odd-permuted, 1/sqrt(HD)-scaled) wq.T
  wkT  [DIM, 128] bf16   permuted wk.T
  wvT  [DIM, 128] bf16   wv.T (not permuted; v is not roped)
  woT  [512, DIM] bf16   wo[:, cols].T
  cosT [128, S]   bf16   [cos; cos] rope table, frequency-major, duplicated
  sinT [128, S]   bf16   [-sin; sin] sign-folded rope table

The per-head even/odd permutation (rows [0,2,..,126,1,3,..,127]) turns RoPE
pair-interleaving into contiguous half-partitions; q.k dot products are
invariant because q and k are permuted identically.

Attention is computed in transposed score layout: scoresT[k, q] so that
probsT feeds the PV matmul directly (lhsT = v natural layout) and attnT
falls out in [hd, q] = exactly the lhsT the output projection needs.

Schedule notes (v3):
  - DMA issue is split across BOTH HWDGE queues: Sync streams xT (dt
    singles first for fast first-arrival, then pairs, depth-2 completion
    chain = 2 transfers in flight); Scalar issues all weights (wk + wq
    head-0 immediately, rest depth-2 chained). PE work can start ~10us in.
  - HAM warm-up matmuls are gated on GpSimd memsets (GpSimd's sequencer
    wakes several us before DVE's), ramping the PE clock during the DMA
    lead-in.
  - Phase A ping-pongs PSUM pools (K/Q1/Q3 -> pool A, Q0/Q2/V -> pools
    At+Op) so each projection's psums are freed by ropes one step ahead of
    the next projection's needs; V-proj is never serialized behind Q3's
    ropes, and pool A is free for the first attention chunk that overlaps
    V-proj.
  - Attention q-chunks run SHORTEST-FIRST [0,512,1024,1536]: the shallow
    chunk-0 pipeline overlaps V-proj, each chunk's output projection
    (pure PE) fills the PE idle of the next chunk's ACT-bound score
    stream, and the final chunk's O-proj is a dense PE-only tail.
  - Engine balance in phase B (PE is the floor at ~125us): exp() is
    ACT-only (~85us); softmax-sum accumulation is split DVE 2/3 : GpSimd
    1/3; O-proj PSUM->SBUF copies are split ACT/DVE per-chunk so neither
    engine exceeds the PE time of its overlap period.
  - PSUM: scores rotate 4 bufs (tag sc), PV accumulators double-buffer
    (tag at), z + O-proj groups share 2 bufs (tag op).
"""

import math
import sys

import numpy as np

try:
    import concourse.bacc as bacc  # noqa: F401
except ImportError:
    sys.path.insert(0, "/opt/trn_rl_repo")

import ml_dtypes
import concourse.bacc as bacc
import concourse.tile as tile
from concourse import mybir
from concourse.bass_utils import run_bass_kernel_spmd
from concourse.bass import _add_dep_helper

BF16 = mybir.dt.bfloat16
F32 = mybir.dt.float32

B, S, DIM = 2, 2048, 2048
H, KVH, HD = 16, 4, 128
N_CORES = 8
P = 128
D_T = DIM // P      # 16 contraction tiles
NH = H // KVH       # 4 q-heads per core
QC = 512            # q-chunk (matmul moving free dim)
QB = S // QC        # 4 q-chunks
S_T = S // P        # 16 s-tiles / k-tiles
N_WARM = 8          # dummy warm-up matmuls to ramp HAM

_cached = {}


def _build_nc():
    nc = bacc.Bacc("TRN2", target_bir_lowering=False, debug=False,
                   num_devices=N_CORES)
    xT = nc.dram_tensor("xT", [DIM, S], BF16, kind="ExternalInput").ap()
    wqp = nc.dram_tensor("wqp", [NH, P, D_T, HD], BF16,
                         kind="ExternalInput").ap()
    wkp = nc.dram_tensor("wkp", [P, D_T, HD], BF16, kind="ExternalInput").ap()
    wvp = nc.dram_tensor("wvp", [P, D_T, HD], BF16, kind="ExternalInput").ap()
    wop = nc.dram_tensor("wop", [P, NH, DIM], BF16, kind="ExternalInput").ap()
    cosT = nc.dram_tensor("cosT", [HD, S], BF16, kind="ExternalInput").ap()
    sinT = nc.dram_tensor("sinT", [HD, S], BF16, kind="ExternalInput").ap()
    out = nc.dram_tensor("out", [S, DIM], BF16, kind="ExternalOutput").ap()

    with tile.TileContext(nc) as tc:
        _build_kernel(tc, xT, wqp, wkp, wvp, wop, cosT, sinT, out)
    nc.compile()
    return nc


def _build_kernel(tc, xT, wqp, wkp, wvp, wop, cosT, sinT, out):
    nc = tc.nc
    Exp = mybir.ActivationFunctionType.Exp

    with (
        tc.tile_pool(name="const", bufs=1) as const,
        tc.tile_pool(name="big", bufs=1) as big,
        tc.tile_pool(name="rtmp", bufs=8) as rtmp,
        tc.tile_pool(name="probs", bufs=9) as probs_pool,
        tc.tile_pool(name="pracc", bufs=3) as pracc_pool,
        tc.tile_pool(name="attn", bufs=6) as attn_pool,
        tc.tile_pool(name="rz", bufs=3) as rz_pool,
        tc.tile_pool(name="osb", bufs=2) as osb_pool,
        tc.tile_pool(name="psA", bufs=4, space="PSUM") as psA,
        tc.tile_pool(name="psAt", bufs=2, space="PSUM") as psAt,
        tc.tile_pool(name="psOp", bufs=2, space="PSUM") as psOp,
    ):
        # ---- constants + HAM warm-up ----
        # memsets on GpSimd: its sequencer wakes earliest, so the dummy
        # matmuls (which only need `dum` initialized) ramp the PE clock
        # governor during the DMA lead-in.
        ones = const.tile([P, P], BF16, name="ones")
        nc.gpsimd.memset(ones, 1.0)
        dum = const.tile([P, QC], BF16, name="dum")
        nc.gpsimd.memset(dum, 0.25)
        warm_ps = psA.tile([P, QC], F32, name="sc")
        for _ in range(N_WARM):
            nc.tensor.matmul(warm_ps[:, 0:384], lhsT=dum[:, 0:P],
                             rhs=dum[:, 0:384], start=True, stop=True)

        # ---- input DMAs ----
        # Sync queue: the xT stream. dt0/dt1 as singles (first data arrives
        # in ~half the time), then pairs; depth-2 completion chain so ~2
        # transfers are in flight and arrive in consumption order.
        xt_sb = big.tile([P, D_T, S], BF16, name="xt")
        xt_tiles = {}
        for dt in range(D_T):
            for sc in range(QB):
                xt_tiles[(dt, sc)] = xt_sb[:, dt, sc * QC:(sc + 1) * QC]
        groups = [(0, 1), (1, 2)] + [(d, d + 2) for d in range(2, D_T, 2)]
        xp_dmas = []
        for gi, (d0, d1) in enumerate(groups):
            dma = nc.sync.dma_start(
                out=xt_sb[:, d0:d1, :],
                in_=xT[d0 * P:d1 * P, :].rearrange("(t p) s -> p t s", p=P))
            if gi >= 2:
                _add_dep_helper(dma.ins, xp_dmas[gi - 2].ins, sync=True,
                                reason="stagger xT load")
            xp_dmas.append(dma)

        # Scalar (ACT) HWDGE queue: weights, all host-pre-arranged to dense
        # per-partition layouts (128 x 4KB+ descriptors, contiguous DRAM
        # source). The 16 DMA engines process descriptors of all transfers
        # in FIFO enqueue order, so late weights are chained onto the xT
        # stream's completions to keep the x tiles strictly ahead.
        wk_sb = big.tile([P, D_T, HD], BF16, name="wk")
        d_wk = nc.scalar.dma_start(out=wk_sb, in_=wkp)
        wq_sb = big.tile([P, NH, D_T, HD], BF16, name="wq")
        d_wq0 = nc.scalar.dma_start(out=wq_sb[:, 0], in_=wqp[0])
        wv_sb = big.tile([P, D_T, HD], BF16, name="wv")
        wo_sb = big.tile([P, NH, DIM], BF16, name="wo")
        cos_sb = const.tile([HD, S], BF16, name="cos")
        sin_sb = const.tile([HD, S], BF16, name="sin")
        for hh in (1, 2, 3):
            d = nc.scalar.dma_start(out=wq_sb[:, hh], in_=wqp[hh])
            _add_dep_helper(d.ins, xp_dmas[hh].ins, sync=True,
                            reason="wq head behind x stream")
        for dst, dsrc, gate in ((cos_sb, cosT, 4), (sin_sb, sinT, 4),
                                (wv_sb, wvp, 6), (wo_sb, wop, 7)):
            d = nc.scalar.dma_start(out=dst, in_=dsrc)
            _add_dep_helper(d.ins, xp_dmas[gate].ins, sync=True,
                            reason="late weight behind x stream")

        qT = big.tile([P, NH, S], BF16, name="qT")
        kT = big.tile([P, S], BF16, name="kT")
        v_sb = big.tile([P, S_T, HD], BF16, name="v")

        def rope(dst, ps, sc):
            """dst (bf16 [128,512] slice) <- rotate(ps).

            ACT stages ps to bf16 SBUF twice (straight + halves swapped via
            ScalarE partition-shifting copies); DVE then runs three
            full-width ops against the sign-folded tables:
            dst = st*[cos;cos] + sw*[-sin;sin]."""
            h = HD // 2
            st = rtmp.tile([P, QC], BF16, name="rst")
            sw = rtmp.tile([P, QC], BF16, name="rsw")
            nc.scalar.copy(out=st, in_=ps)
            nc.scalar.copy(out=sw[0:h, :], in_=ps[h:P, :])
            nc.scalar.copy(out=sw[h:P, :], in_=ps[0:h, :])
            cos_c = cos_sb[:, sc * QC:(sc + 1) * QC]
            sin_c = sin_sb[:, sc * QC:(sc + 1) * QC]
            t0 = rtmp.tile([P, QC], BF16, name="rt")
            t1 = rtmp.tile([P, QC], BF16, name="rt")
            nc.vector.tensor_mul(t0, st, cos_c)
            nc.vector.tensor_mul(t1, sw, sin_c)
            nc.vector.tensor_add(dst, t0, t1)

        # ---- K projection + Q head-0, dt-outer ----
        # K runs 4 dt-tiles ahead of Q-h0 so the PE starts as soon as the
        # first xT tile lands (wq0 arrives about then on the other queue).
        # K -> pool A, Q0 -> pools At+Op.
        kps = [psA.tile([P, QC], F32, name="sc") for _ in range(QB)]
        q0ps = [psAt.tile([P, QC], F32, name="at"),
                psAt.tile([P, QC], F32, name="at"),
                psOp.tile([P, QC], F32, name="op"),
                psOp.tile([P, QC], F32, name="op")]

        def kmm(dt):
            for sc in range(QB):
                nc.tensor.matmul(kps[sc], lhsT=wk_sb[:, dt, :],
                                 rhs=xt_tiles[(dt, sc)],
                                 start=(dt == 0), stop=(dt == D_T - 1))

        def q0mm(dt):
            for sc in range(QB):
                nc.tensor.matmul(q0ps[sc], lhsT=wq_sb[:, 0, dt, :],
                                 rhs=xt_tiles[(dt, sc)],
                                 start=(dt == 0), stop=(dt == D_T - 1))

        for dt in range(4):
            kmm(dt)
        for dt in range(4, D_T):
            kmm(dt)
            q0mm(dt - 4)
        for dt in range(D_T - 4, D_T):
            q0mm(dt)

        # K ropes first: K's psums (pool A, which Q1 needs) finished 4
        # dt-steps before q0's, so they drain while q0's last matmuls run.
        for sc in range(QB):
            rope(kT[:, sc * QC:(sc + 1) * QC], kps[sc], sc)
        for sc in range(QB):
            rope(qT[:, 0, sc * QC:(sc + 1) * QC], q0ps[sc], sc)

        # ---- Q heads 1..3, dt-outer per head, ping-ponging pools ----
        for hh in range(1, NH):
            if hh % 2 == 1:
                qps = [psA.tile([P, QC], F32, name="sc") for _ in range(QB)]
            else:
                qps = [psAt.tile([P, QC], F32, name="at"),
                       psAt.tile([P, QC], F32, name="at"),
                       psOp.tile([P, QC], F32, name="op"),
                       psOp.tile([P, QC], F32, name="op")]
            for dt in range(D_T):
                for sc in range(QB):
                    nc.tensor.matmul(
                        qps[sc], lhsT=wq_sb[:, hh, dt, :],
                        rhs=xt_tiles[(dt, sc)],
                        start=(dt == 0), stop=(dt == D_T - 1))
            for sc in range(QB):
                rope(qT[:, hh, sc * QC:(sc + 1) * QC], qps[sc], sc)

        # ---- V projection (natural [s, hd] layout) ----
        # Pools At/Op (freed by Q2's ropes long before); pool A stays free
        # for the chunk-0 attention stream that overlaps V-proj.
        for st in range(S_T):
            if st % 2 == 0:
                ps = psAt.tile([P, QC], F32, name="at")
            else:
                ps = psOp.tile([P, QC], F32, name="op")
            for dt in range(D_T):
                nc.tensor.matmul(
                    ps[:, 0:HD],
                    lhsT=xt_tiles[(dt, st // 4)][:, (st % 4) * P:(st % 4 + 1) * P],
                    rhs=wv_sb[:, dt, :],
                    start=(dt == 0), stop=(dt == D_T - 1))
            nc.vector.tensor_copy(out=v_sb[:, st, :], in_=ps[:, 0:HD])

        # ---- attention + output projection, per q-chunk ----
        # Chunks run SHORTEST-first: the shallow chunk-0 pipeline overlaps
        # V-proj, each chunk's O-proj (pure PE) fills the PE idle time of
        # the next chunk's ACT-bound score stream, and the final (longest)
        # chunk leaves a dense PE-only O-proj tail.
        # O-proj copy engine split per chunk: the fraction on ACT shrinks
        # when the overlapping score stream is ACT-heavy.
        chunks = [(0, 512), (512, 512), (1024, 512), (1536, 512)]
        # index (st*4+dc)%4 -> True=ACT copy, False=DVE copy, per chunk
        copy_act = {
            0: (True, True, True, False),   # overlaps chunk-512 stream
            1: (True, True, False, False),  # overlaps chunk-1024 stream
            2: (True, False, False, False),  # overlaps chunk-1536 (ACT-hot)
            3: (True, False, True, False),  # standalone tail
        }
        for ci, (q0, qw) in enumerate(chunks):
            nk = (q0 + qw) // P  # causal k-tiles for this q-chunk
            attn_tiles = []
            for hh in range(NH):
                at_ps = psAt.tile([P, qw], F32, name="at")
                pr_acc = pracc_pool.tile([P, qw], BF16, name="pracc")
                for k in range(nk):
                    # On diagonal tiles only columns q0+off.. are causally
                    # valid; narrow every stage to that width.
                    off = max(0, k * P - q0)
                    w = qw - off
                    diag = k * P >= q0
                    sc_ps = psA.tile([P, QC], F32, name="sc")
                    nc.tensor.matmul(sc_ps[:, 0:w], lhsT=kT[:, k * P:(k + 1) * P],
                                     rhs=qT[:, hh, q0 + off:q0 + qw],
                                     start=True, stop=True)
                    pr = probs_pool.tile([P, QC], BF16, name="pr")
                    nc.scalar.activation(out=pr[:, 0:w], in_=sc_ps[:, 0:w],
                                         func=Exp)
                    if diag:  # zero where c' < r
                        nc.gpsimd.affine_select(
                            out=pr[:, 0:w], in_=pr[:, 0:w],
                            compare_op=mybir.AluOpType.is_ge,
                            fill=0.0, base=0, pattern=[[1, w]],
                            channel_multiplier=-1)
                    nc.tensor.matmul(at_ps[:, off:qw], lhsT=v_sb[:, k, :],
                                     rhs=pr[:, 0:w],
                                     start=(k == 0), stop=(k == nk - 1))
                    if k == 0:
                        nc.vector.tensor_copy(out=pr_acc, in_=pr[:, 0:qw])
                    else:
                        nc.vector.tensor_add(pr_acc[:, off:qw],
                                             pr_acc[:, off:qw], pr[:, 0:w])
                z_ps = psOp.tile([P, qw], F32, name="op")
                nc.tensor.matmul(z_ps, lhsT=ones, rhs=pr_acc,
                                 start=True, stop=True)
                rz = rz_pool.tile([P, qw], F32, name="rz")
                nc.vector.reciprocal_approx_fast(out=rz, in_=z_ps)
                a_sb = attn_pool.tile([P, qw], BF16, name="attn")
                nc.vector.tensor_mul(a_sb, at_ps, rz)
                attn_tiles.append(a_sb)

            # Output projection for this chunk; all output DMAs issue from
            # Sync (idle in phase B; ACT must stay mostly exp-only).
            for st in range(qw // P):
                row0 = q0 + st * P
                o_sb = osb_pool.tile([P, DIM], BF16, name="osb")
                for dc in range(DIM // QC):
                    op_ps = psOp.tile([P, QC], F32, name="op")
                    for j in range(NH):
                        nc.tensor.matmul(
                            op_ps, lhsT=attn_tiles[j][:, st * P:(st + 1) * P],
                            rhs=wo_sb[:, j, dc * QC:(dc + 1) * QC],
                            start=(j == 0), stop=(j == NH - 1))
                    if copy_act[ci][(st * 4 + dc) % 4]:
                        nc.scalar.copy(out=o_sb[:, dc * QC:(dc + 1) * QC],
                                       in_=op_ps)
                    else:
                        nc.vector.tensor_copy(out=o_sb[:, dc * QC:(dc + 1) * QC],
                                              in_=op_ps)
                    if dc == 1:
                        nc.sync.dma_start(out=out[row0:row0 + P, 0:2 * QC],
                                          in_=o_sb[:, 0:2 * QC])
                nc.sync.dma_start(out=out[row0:row0 + P, 2 * QC:DIM],
                                  in_=o_sb[:, 2 * QC:DIM])


def _get_nc():
    if "nc" not in _cached:
        _cached["nc"] = _build_nc()
    return _cached["nc"]


def _prep_in_maps(x, freqs_cis, wq, wk, wv, wo):
    bf = ml_dtypes.bfloat16
    perm = np.concatenate([np.arange(0, HD, 2), np.arange(1, HD, 2)])
    scale = 1.0 / math.sqrt(HD)
    wq_p = (wq.reshape(H, HD, DIM)[:, perm, :] * scale).astype(np.float32)
    wk_p = wk.reshape(KVH, HD, DIM)[:, perm, :]
    cos_h = np.ascontiguousarray(freqs_cis[:, :, 0].T)  # [64, S]
    sin_h = np.ascontiguousarray(freqs_cis[:, :, 1].T)
    cosT = np.concatenate([cos_h, cos_h], axis=0).astype(bf)   # [128, S]
    sinT = np.concatenate([-sin_h, sin_h], axis=0).astype(bf)

    in_maps = []
    for c in range(N_CORES):
        b, g = c // KVH, c % KVH
        hq = slice(NH * g, NH * (g + 1))
        def p_t_j(wT):  # [DIM, J] -> [P, D_T, J] dense per partition
            J = wT.shape[1]
            return np.ascontiguousarray(
                wT.reshape(D_T, P, J).transpose(1, 0, 2)).astype(bf)

        wq_core = wq_p[hq].reshape(NH * HD, DIM).T  # [DIM, NH*HD]
        wqp_h = np.ascontiguousarray(
            wq_core.reshape(D_T, P, NH, HD).transpose(2, 1, 0, 3)).astype(bf)
        wo_core = wo[:, NH * HD * g:NH * HD * (g + 1)].T  # [NH*HD, DIM]
        wop_h = np.ascontiguousarray(
            wo_core.reshape(NH, HD, DIM).transpose(1, 0, 2)).astype(bf)
        in_maps.append({
            "xT": np.ascontiguousarray(x[b].T).astype(bf),
            "wqp": wqp_h,
            "wkp": p_t_j(np.ascontiguousarray(wk_p[g].T)),
            "wvp": p_t_j(np.ascontiguousarray(wv[g * HD:(g + 1) * HD].T)),
            "wop": wop_h,
            "cosT": cosT,
            "sinT": sinT,
        })
    return in_maps


def _reduce_outputs(results):
    out = np.zeros((B, S, DIM), np.float32)
    for c in range(N_CORES):
        out[c // KVH] += results[c]["out"].astype(np.float32)
    return out


def kernel(x, freqs_cis, wq, wk, wv, wo, _trace=False, _trace_kwargs=None):
    nc = _get_nc()
    x, freqs_cis, wq, wk, wv, wo = (
        np.asarray(a, np.float32) for a in (x, freqs_cis, wq, wk, wv, wo))
    in_maps = _prep_in_maps(x, freqs_cis, wq, wk, wv, wo)
    res = run_bass_kernel_spmd(nc, in_maps, core_ids=list(range(N_CORES)),
                               trace=_trace, **(_trace_kwargs or {}))
    out = _reduce_outputs(res.results)
    if _trace:
        _cached["last_exec_time_ns"] = res.exec_time_ns
        _cached["last_results"] = res
    return out


# revision 10
# speedup vs baseline: 1.0582x; 1.0342x over previous
"""GQA attention (B=2, S=2048, DIM=2048, H=16, KVH=4, HD=128, RoPE, causal)
on 8 TRN2 NeuronCores.

Sharding: core c -> batch b = c//4, head-group g = c%4 (q heads 4g..4g+3,
which map exactly to kv head g). Each core computes the partial output
attn_heads @ wo_slice.T  ([S, DIM]); the host sums the 4 partials per batch.

Device layout (everything "transposed", feature-major; DRAM sources are kept
contiguous in transfer order — strided per-partition DRAM layouts measure
10-30x slower per descriptor on HW):
  xT   [DIM, S]   bf16   x[b].T
  wqT  [DIM, 512] bf16   (per-head even/odd-permuted, 1/sqrt(HD)-scaled) wq.T
  wkT  [DIM, 128] bf16   permuted wk.T
  wvT  [DIM, 128] bf16   wv.T (not permuted; v is not roped)
  woT  [512, DIM] bf16   wo[:, cols].T
  cosT [128, S]   bf16   [cos; cos] rope table, frequency-major, duplicated
  sinT [128, S]   bf16   [-sin; sin] sign-folded rope table

The per-head even/odd permutation (rows [0,2,..,126,1,3,..,127]) turns RoPE
pair-interleaving into contiguous half-partitions; q.k dot products are
invariant because q and k are permuted identically.

Attention is computed in transposed score layout: scoresT[k, q] so that
probsT feeds the PV matmul directly (lhsT = v natural layout) and attnT
falls out in [hd, q] = exactly the lhsT the output projection needs.

Schedule notes:
  - DMA issue is split across BOTH HWDGE queues: Sync streams xT (dt
    singles first for fast first-arrival, then pairs, depth-2 completion
    chain = 2 transfers in flight); Scalar issues all weights as dense
    per-partition pre-arranged transfers (128 x 4KB+ descriptors), with
    late weights chained onto xT completions so the x stream stays ahead
    in the DMA engines' FIFO. PE work starts ~11us in.
  - HAM warm-up matmuls are gated on GpSimd memsets (GpSimd's sequencer
    wakes several us before DVE's), ramping the PE clock during the DMA
    lead-in.
  - Phase A ping-pongs PSUM pools (K/Q1/Q3 -> pool A, Q0/Q2/V -> pools
    At+Op) so each projection's psums are freed by ropes one step ahead
    of the next projection's needs.
  - Attention q-chunks run SHORTEST-FIRST [0,512,1024,1536]: the shallow
    chunk-0 pipeline overlaps V-proj and each chunk's output projection
    is emitted as a thunk list drained INTERLEAVED into the next chunk's
    score stream (the OOO scheduler alone would run the earlier-emitted
    O-proj block first, leaving the ACT-bound exp stream to stall the
    PE). The final chunk's O-proj is a dense PE-only tail.
  - Engine balance in phase B (PE is the floor): exp() is ACT-only;
    softmax-sum accumulation and normalize on DVE; masks on GpSimd;
    O-proj PSUM->SBUF copies split ACT/DVE per chunk so neither engine
    exceeds the PE time of its overlap period; output DMAs issue from
    Sync only.
  - PSUM: scores rotate 4 bufs (tag sc), PV accumulators double-buffer
    (tag at), z + O-proj groups share 2 bufs (tag op).
"""

import math
import sys

import numpy as np

try:
    import concourse.bacc as bacc  # noqa: F401
except ImportError:
    sys.path.insert(0, "/opt/trn_rl_repo")

import ml_dtypes
import concourse.bacc as bacc
import concourse.tile as tile
from concourse import mybir
from concourse.bass_utils import run_bass_kernel_spmd
from concourse.bass import _add_dep_helper

BF16 = mybir.dt.bfloat16
F32 = mybir.dt.float32

B, S, DIM = 2, 2048, 2048
H, KVH, HD = 16, 4, 128
N_CORES = 8
P = 128
D_T = DIM // P      # 16 contraction tiles
NH = H // KVH       # 4 q-heads per core
QC = 512            # q-chunk (matmul moving free dim)
QB = S // QC        # 4 q-chunks
S_T = S // P        # 16 s-tiles / k-tiles
N_WARM = 8          # dummy warm-up matmuls to ramp HAM

_cached = {}


def _build_nc():
    nc = bacc.Bacc("TRN2", target_bir_lowering=False, debug=False,
                   num_devices=N_CORES)
    xT = nc.dram_tensor("xT", [DIM, S], BF16, kind="ExternalInput").ap()
    wqp = nc.dram_tensor("wqp", [NH, P, D_T, HD], BF16,
                         kind="ExternalInput").ap()
    wkp = nc.dram_tensor("wkp", [P, D_T, HD], BF16, kind="ExternalInput").ap()
    wvp = nc.dram_tensor("wvp", [P, D_T, HD], BF16, kind="ExternalInput").ap()
    wop = nc.dram_tensor("wop", [P, NH, DIM], BF16, kind="ExternalInput").ap()
    cosT = nc.dram_tensor("cosT", [HD, S], BF16, kind="ExternalInput").ap()
    sinT = nc.dram_tensor("sinT", [HD, S], BF16, kind="ExternalInput").ap()
    out = nc.dram_tensor("out", [S, DIM], BF16, kind="ExternalOutput").ap()

    with tile.TileContext(nc) as tc:
        _build_kernel(tc, xT, wqp, wkp, wvp, wop, cosT, sinT, out)
    nc.compile()
    return nc


def _build_kernel(tc, xT, wqp, wkp, wvp, wop, cosT, sinT, out):
    nc = tc.nc
    Exp = mybir.ActivationFunctionType.Exp

    with (
        tc.tile_pool(name="const", bufs=1) as const,
        tc.tile_pool(name="big", bufs=1) as big,
        tc.tile_pool(name="rtmp", bufs=8) as rtmp,
        tc.tile_pool(name="probs", bufs=9) as probs_pool,
        tc.tile_pool(name="pracc", bufs=3) as pracc_pool,
        tc.tile_pool(name="attn", bufs=6) as attn_pool,
        tc.tile_pool(name="rz", bufs=3) as rz_pool,
        tc.tile_pool(name="osb", bufs=2) as osb_pool,
        tc.tile_pool(name="psA", bufs=4, space="PSUM") as psA,
        tc.tile_pool(name="psAt", bufs=2, space="PSUM") as psAt,
        tc.tile_pool(name="psOp", bufs=2, space="PSUM") as psOp,
    ):
        # ---- constants + HAM warm-up ----
        # memsets on GpSimd: its sequencer wakes earliest, so the dummy
        # matmuls (which only need `dum` initialized) ramp the PE clock
        # governor during the DMA lead-in.
        ones = const.tile([P, P], BF16, name="ones")
        nc.gpsimd.memset(ones, 1.0)
        dum = const.tile([P, QC], BF16, name="dum")
        nc.gpsimd.memset(dum, 0.25)
        warm_ps = psA.tile([P, QC], F32, name="sc")
        for _ in range(N_WARM):
            nc.tensor.matmul(warm_ps[:, 0:384], lhsT=dum[:, 0:P],
                             rhs=dum[:, 0:384], start=True, stop=True)

        # ---- input DMAs ----
        # Sync queue: the xT stream. dt0/dt1 as singles (first data arrives
        # in ~half the time), then pairs; depth-2 completion chain so ~2
        # transfers are in flight and arrive in consumption order.
        xt_sb = big.tile([P, D_T, S], BF16, name="xt")
        xt_tiles = {}
        for dt in range(D_T):
            for sc in range(QB):
                xt_tiles[(dt, sc)] = xt_sb[:, dt, sc * QC:(sc + 1) * QC]
        groups = [(0, 1), (1, 2)] + [(d, d + 2) for d in range(2, D_T, 2)]
        xp_dmas = []
        for gi, (d0, d1) in enumerate(groups):
            dma = nc.sync.dma_start(
                out=xt_sb[:, d0:d1, :],
                in_=xT[d0 * P:d1 * P, :].rearrange("(t p) s -> p t s", p=P))
            if gi >= 2:
                _add_dep_helper(dma.ins, xp_dmas[gi - 2].ins, sync=True,
                                reason="stagger xT load")
            xp_dmas.append(dma)

        # Scalar (ACT) HWDGE queue: weights, all host-pre-arranged to dense
        # per-partition layouts (128 x 4KB+ descriptors, contiguous DRAM
        # source). The 16 DMA engines process descriptors of all transfers
        # in FIFO enqueue order, so late weights are chained onto the xT
        # stream's completions to keep the x tiles strictly ahead.
        wk_sb = big.tile([P, D_T, HD], BF16, name="wk")
        d_wk = nc.scalar.dma_start(out=wk_sb, in_=wkp)
        wq_sb = big.tile([P, NH, D_T, HD], BF16, name="wq")
        d_wq0 = nc.scalar.dma_start(out=wq_sb[:, 0], in_=wqp[0])
        wv_sb = big.tile([P, D_T, HD], BF16, name="wv")
        wo_sb = big.tile([P, NH, DIM], BF16, name="wo")
        cos_sb = const.tile([HD, S], BF16, name="cos")
        sin_sb = const.tile([HD, S], BF16, name="sin")
        for hh in (1, 2, 3):
            d = nc.scalar.dma_start(out=wq_sb[:, hh], in_=wqp[hh])
            _add_dep_helper(d.ins, xp_dmas[hh].ins, sync=True,
                            reason="wq head behind x stream")
        for dst, dsrc, gate in ((cos_sb, cosT, 4), (sin_sb, sinT, 4),
                                (wv_sb, wvp, 6), (wo_sb, wop, 7)):
            d = nc.scalar.dma_start(out=dst, in_=dsrc)
            _add_dep_helper(d.ins, xp_dmas[gate].ins, sync=True,
                            reason="late weight behind x stream")

        qT = big.tile([P, NH, S], BF16, name="qT")
        kT = big.tile([P, S], BF16, name="kT")
        v_sb = big.tile([P, S_T, HD], BF16, name="v")

        def rope(dst, ps, sc):
            """dst (bf16 [128,512] slice) <- rotate(ps).

            ACT stages ps to bf16 SBUF twice (straight + halves swapped via
            ScalarE partition-shifting copies); DVE then runs three
            full-width ops against the sign-folded tables:
            dst = st*[cos;cos] + sw*[-sin;sin]."""
            h = HD // 2
            st = rtmp.tile([P, QC], BF16, name="rst")
            sw = rtmp.tile([P, QC], BF16, name="rsw")
            nc.scalar.copy(out=st, in_=ps)
            nc.scalar.copy(out=sw[0:h, :], in_=ps[h:P, :])
            nc.scalar.copy(out=sw[h:P, :], in_=ps[0:h, :])
            cos_c = cos_sb[:, sc * QC:(sc + 1) * QC]
            sin_c = sin_sb[:, sc * QC:(sc + 1) * QC]
            t0 = rtmp.tile([P, QC], BF16, name="rt")
            t1 = rtmp.tile([P, QC], BF16, name="rt")
            nc.vector.tensor_mul(t0, st, cos_c)
            nc.vector.tensor_mul(t1, sw, sin_c)
            nc.vector.tensor_add(dst, t0, t1)

        # ---- K projection + Q head-0, dt-outer ----
        # K runs 4 dt-tiles ahead of Q-h0 so the PE starts as soon as the
        # first xT tile lands (wq0 arrives about then on the other queue).
        # K -> pool A, Q0 -> pools At+Op.
        kps = [psA.tile([P, QC], F32, name="sc") for _ in range(QB)]
        q0ps = [psAt.tile([P, QC], F32, name="at"),
                psAt.tile([P, QC], F32, name="at"),
                psOp.tile([P, QC], F32, name="op"),
                psOp.tile([P, QC], F32, name="op")]

        def kmm(dt):
            for sc in range(QB):
                nc.tensor.matmul(kps[sc], lhsT=wk_sb[:, dt, :],
                                 rhs=xt_tiles[(dt, sc)],
                                 start=(dt == 0), stop=(dt == D_T - 1))

        def q0mm(dt):
            for sc in range(QB):
                nc.tensor.matmul(q0ps[sc], lhsT=wq_sb[:, 0, dt, :],
                                 rhs=xt_tiles[(dt, sc)],
                                 start=(dt == 0), stop=(dt == D_T - 1))

        for dt in range(4):
            kmm(dt)
        for dt in range(4, D_T):
            kmm(dt)
            q0mm(dt - 4)
        for dt in range(D_T - 4, D_T):
            q0mm(dt)

        # K ropes first: K's psums (pool A, which Q1 needs) finished 4
        # dt-steps before q0's, so they drain while q0's last matmuls run.
        for sc in range(QB):
            rope(kT[:, sc * QC:(sc + 1) * QC], kps[sc], sc)
        for sc in range(QB):
            rope(qT[:, 0, sc * QC:(sc + 1) * QC], q0ps[sc], sc)

        # ---- Q heads 1..3, dt-outer per head, ping-ponging pools ----
        for hh in range(1, NH):
            if hh % 2 == 1:
                qps = [psA.tile([P, QC], F32, name="sc") for _ in range(QB)]
            else:
                qps = [psAt.tile([P, QC], F32, name="at"),
                       psAt.tile([P, QC], F32, name="at"),
                       psOp.tile([P, QC], F32, name="op"),
                       psOp.tile([P, QC], F32, name="op")]
            for dt in range(D_T):
                for sc in range(QB):
                    nc.tensor.matmul(
                        qps[sc], lhsT=wq_sb[:, hh, dt, :],
                        rhs=xt_tiles[(dt, sc)],
                        start=(dt == 0), stop=(dt == D_T - 1))
            for sc in range(QB):
                rope(qT[:, hh, sc * QC:(sc + 1) * QC], qps[sc], sc)

        # ---- V projection (natural [s, hd] layout) ----
        # Pools At/Op (freed by Q2's ropes long before); pool A stays free
        # for the chunk-0 attention stream that overlaps V-proj.
        for st in range(S_T):
            if st % 2 == 0:
                ps = psAt.tile([P, QC], F32, name="at")
            else:
                ps = psOp.tile([P, QC], F32, name="op")
            for dt in range(D_T):
                nc.tensor.matmul(
                    ps[:, 0:HD],
                    lhsT=xt_tiles[(dt, st // 4)][:, (st % 4) * P:(st % 4 + 1) * P],
                    rhs=wv_sb[:, dt, :],
                    start=(dt == 0), stop=(dt == D_T - 1))
            nc.vector.tensor_copy(out=v_sb[:, st, :], in_=ps[:, 0:HD])

        # ---- attention + output projection, per q-chunk ----
        # Chunks run SHORTEST-first: the shallow chunk-0 pipeline overlaps
        # V-proj, each chunk's O-proj (pure PE) fills the PE idle time of
        # the next chunk's ACT-bound score stream, and the final (longest)
        # chunk leaves a dense PE-only O-proj tail.
        # O-proj copy engine split per chunk: the fraction on ACT shrinks
        # when the overlapping score stream is ACT-heavy.
        chunks = [(0, 512), (512, 512), (1024, 512), (1536, 512)]
        # index (st*4+dc)%4 -> True=ACT copy, False=DVE copy, per chunk
        copy_act = {
            0: (True, True, True, False),   # overlaps chunk-512 stream
            1: (True, True, False, False),  # overlaps chunk-1024 stream
            2: (True, False, False, False),  # overlaps chunk-1536 (ACT-hot)
            3: (True, False, True, False),  # standalone tail
        }
        for ci, (q0, qw) in enumerate(chunks):
            nk = (q0 + qw) // P  # causal k-tiles for this q-chunk
            attn_tiles = []
            for hh in range(NH):
                at_ps = psAt.tile([P, qw], F32, name="at")
                pr_acc = pracc_pool.tile([P, qw], BF16, name="pracc")
                for k in range(nk):
                    # On diagonal tiles only columns q0+off.. are causally
                    # valid; narrow every stage to that width.
                    off = max(0, k * P - q0)
                    w = qw - off
                    diag = k * P >= q0
                    sc_ps = psA.tile([P, QC], F32, name="sc")
                    nc.tensor.matmul(sc_ps[:, 0:w], lhsT=kT[:, k * P:(k + 1) * P],
                                     rhs=qT[:, hh, q0 + off:q0 + qw],
                                     start=True, stop=True)
                    pr = probs_pool.tile([P, QC], BF16, name="pr")
                    nc.scalar.activation(out=pr[:, 0:w], in_=sc_ps[:, 0:w],
                                         func=Exp)
                    if diag:  # zero where c' < r
                        nc.gpsimd.affine_select(
                            out=pr[:, 0:w], in_=pr[:, 0:w],
                            compare_op=mybir.AluOpType.is_ge,
                            fill=0.0, base=0, pattern=[[1, w]],
                            channel_multiplier=-1)
                    nc.tensor.matmul(at_ps[:, off:qw], lhsT=v_sb[:, k, :],
                                     rhs=pr[:, 0:w],
                                     start=(k == 0), stop=(k == nk - 1))
                    if k == 0:
                        nc.vector.tensor_copy(out=pr_acc, in_=pr[:, 0:qw])
                    else:
                        nc.vector.tensor_add(pr_acc[:, off:qw],
                                             pr_acc[:, off:qw], pr[:, 0:w])
                z_ps = psOp.tile([P, qw], F32, name="op")
                nc.tensor.matmul(z_ps, lhsT=ones, rhs=pr_acc,
                                 start=True, stop=True)
                rz = rz_pool.tile([P, qw], F32, name="rz")
                nc.vector.reciprocal_approx_fast(out=rz, in_=z_ps)
                a_sb = attn_pool.tile([P, qw], BF16, name="attn")
                nc.vector.tensor_mul(a_sb, at_ps, rz)
                attn_tiles.append(a_sb)

            # Output projection for this chunk; all output DMAs issue from
            # Sync (idle in phase B; ACT must stay mostly exp-only).
            for st in range(qw // P):
                row0 = q0 + st * P
                o_sb = osb_pool.tile([P, DIM], BF16, name="osb")
                for dc in range(DIM // QC):
                    op_ps = psOp.tile([P, QC], F32, name="op")
                    for j in range(NH):
                        nc.tensor.matmul(
                            op_ps, lhsT=attn_tiles[j][:, st * P:(st + 1) * P],
                            rhs=wo_sb[:, j, dc * QC:(dc + 1) * QC],
                            start=(j == 0), stop=(j == NH - 1))
                    if copy_act[ci][(st * 4 + dc) % 4]:
                        nc.scalar.copy(out=o_sb[:, dc * QC:(dc + 1) * QC],
                                       in_=op_ps)
                    else:
                        nc.vector.tensor_copy(out=o_sb[:, dc * QC:(dc + 1) * QC],
                                              in_=op_ps)
                    if dc == 1:
                        nc.sync.dma_start(out=out[row0:row0 + P, 0:2 * QC],
                                          in_=o_sb[:, 0:2 * QC])
                nc.sync.dma_start(out=out[row0:row0 + P, 2 * QC:DIM],
                                  in_=o_sb[:, 2 * QC:DIM])


def _get_nc():
    if "nc" not in _cached:
        _cached["nc"] = _build_nc()
    return _cached["nc"]


def _prep_in_maps(x, freqs_cis, wq, wk, wv, wo):
    bf = ml_dtypes.bfloat16
    perm = np.concatenate([np.arange(0, HD, 2), np.arange(1, HD, 2)])
    scale = 1.0 / math.sqrt(HD)
    wq_p = (wq.reshape(H, HD, DIM)[:, perm, :] * scale).astype(np.float32)
    wk_p = wk.reshape(KVH, HD, DIM)[:, perm, :]
    cos_h = np.ascontiguousarray(freqs_cis[:, :, 0].T)  # [64, S]
    sin_h = np.ascontiguousarray(freqs_cis[:, :, 1].T)
    cosT = np.concatenate([cos_h, cos_h], axis=0).astype(bf)   # [128, S]
    sinT = np.concatenate([-sin_h, sin_h], axis=0).astype(bf)

    in_maps = []
    for c in range(N_CORES):
        b, g = c // KVH, c % KVH
        hq = slice(NH * g, NH * (g + 1))
        def p_t_j(wT):  # [DIM, J] -> [P, D_T, J] dense per partition
            J = wT.shape[1]
            return np.ascontiguousarray(
                wT.reshape(D_T, P, J).transpose(1, 0, 2)).astype(bf)

        wq_core = wq_p[hq].reshape(NH * HD, DIM).T  # [DIM, NH*HD]
        wqp_h = np.ascontiguousarray(
            wq_core.reshape(D_T, P, NH, HD).transpose(2, 1, 0, 3)).astype(bf)
        wo_core = wo[:, NH * HD * g:NH * HD * (g + 1)].T  # [NH*HD, DIM]
        wop_h = np.ascontiguousarray(
            wo_core.reshape(NH, HD, DIM).transpose(1, 0, 2)).astype(bf)
        in_maps.append({
            "xT": np.ascontiguousarray(x[b].T).astype(bf),
            "wqp": wqp_h,
            "wkp": p_t_j(np.ascontiguousarray(wk_p[g].T)),
            "wvp": p_t_j(np.ascontiguousarray(wv[g * HD:(g + 1) * HD].T)),
            "wop": wop_h,
            "cosT": cosT,
            "sinT": sinT,
        })
    return in_maps


def _reduce_outputs(results):
    out = np.zeros((B, S, DIM), np.float32)
    for c in range(N_CORES):
        out[c // KVH] += results[c]["out"].astype(np.float32)
    return out


def kernel(x, freqs_cis, wq, wk, wv, wo, _trace=False, _trace_kwargs=None):
    nc = _get_nc()
    x, freqs_cis, wq, wk, wv, wo = (
        np.asarray(a, np.float32) for a in (x, freqs_cis, wq, wk, wv, wo))
    in_maps = _prep_in_maps(x, freqs_cis, wq, wk, wv, wo)
    res = run_bass_kernel_spmd(nc, in_maps, core_ids=list(range(N_CORES)),
                               trace=_trace, **(_trace_kwargs or {}))
    out = _reduce_outputs(res.results)
    if _trace:
        _cached["last_exec_time_ns"] = res.exec_time_ns
        _cached["last_results"] = res
    return out
